# revision 47
# baseline (speedup 1.0000x reference)
"""Trainium2 Bass kernel for two-stage pooled-query attention.

Problem (hardcoded):
    B=32, N=577, C=1024, H=16 heads, d=64, pooled queries 8x8 (3x3 mean over
    24x24 grid of non-cls tokens).
    qkv = X @ W_qkv.T ; pool Xq -> Qp ; s1 = softmax(Qp*s @ K^T) @ V ;
    s2 = softmax(Xq*s @ Qp^T) @ s1 ; out = s2 @ W_proj.T + b_proj

Strategy: pure data-parallel over batch across 8 NeuronCores (4 batches per
core, no collectives). The big QKV GEMM runs fp8 DoubleRow (K=256 per
instruction at 0.5 cycles/row) with a 3-term hi/lo hybrid split to hold
accuracy:
  X @ W ~= X_hi@W_hi + X_lo@W_hi + X_hi@W_lo, each operand split on the host
  into an e4m3 value plus an e4m3 residual (W_qkv pre-scaled by 32 so its
  residual clears the fp8 subnormal floor; the 32 cancels through the pooled
  query scale and W_proj/32).

The attention middle runs bf16 with no PE transposes:
  - Stage 1 is computed transposed, s1T [n, 2q] = qkT_k.T @ QpBD, so the
    A1 weights land directly in the lhsT layout Qd needs. The softmax
    denominator Z1 comes from a 1-column ones matmul (~free), and 1/Z1 is
    folded per-q-partition into stage 2's weights instead of into Qd.
  - Stage 2 is computed transposed too, E2T [2q, n] = exp(QpBD.T @ qkT_q),
    the per-(token,head) denominator is produced pre-broadcast by one
    block-diag-ones matmul, and a single fused scalar_tensor_tensor gives
    e2n = E2T * r1[q] * (1/Z2) -- both normalizations in one op. The output
    outT [c, n] = QdBD_raw @ e2n needs no A2 transpose either.

Batches are software-pipelined: batch b+1's QKV GEMM instructions are
emitted interleaved into batch b's attention middle, so the in-order PE
queue always has independent work while the middle waits on softmax.
"""

import os
import sys

import numpy as np

sys.path.insert(0, "/opt/trn_rl_repo")

import ml_dtypes  # noqa: E402

import concourse.tile as tile  # noqa: E402
from concourse import bacc, mybir  # noqa: E402
from concourse.bass_utils import run_bass_kernel_spmd  # noqa: E402

B, N, C = 32, 577, 1024
H, D = 16, 64
SCALE = D ** -0.5
N_CORES = 8
NB = B // N_CORES  # batches per core

BF16 = mybir.dt.bfloat16
F32 = mybir.dt.float32
FP8 = mybir.dt.float8e4
DR = mybir.MatmulPerfMode.DoubleRow
WSCALE = 32.0  # host pre-scale on W_qkv^T (keeps fp8 residuals normal)
MULT = mybir.AluOpType.mult

# token chunks of 577 = 4*128 + 65
TOK = [(0, 128), (128, 128), (256, 128), (384, 128), (512, 65)]
# free-dim chunks of 577 for wide matmuls: one full psum bank + a stub
NF = [(0, 512), (512, 65)]
EXP = mybir.ActivationFunctionType.Exp


def build_program(nb: int = NB):
    nc = bacc.Bacc("TRN2", target_bir_lowering=False, debug=False)

    # host pre-arranges operands into the exact SBUF tile layout
    # [c2, p, i(sub-chunk), v(hi/lo), cols] so every DMA is contiguous
    x8_d = nc.dram_tensor("x8", [nb, 4, 128, 2, 2, 640], FP8, kind="ExternalInput")
    w8_d = nc.dram_tensor("w8", [4, 128, 2, 2, 3 * C], FP8, kind="ExternalInput")
    wp8_d = nc.dram_tensor("wp8", [4, 128, 2, 2, C], FP8, kind="ExternalInput")
    wbias_d = nc.dram_tensor("wbias", [1, C], BF16, kind="ExternalInput")
    out_d = nc.dram_tensor("out", [nb, N, C], BF16, kind="ExternalOutput")

    with tile.TileContext(nc) as tc:
        const_pool = tc.alloc_tile_pool(name="const", bufs=1)
        w_pool = tc.alloc_tile_pool(name="w", bufs=1)
        sb = tc.alloc_tile_pool(name="sb", bufs=2)
        ps_big = tc.alloc_tile_pool(name="ps_big", bufs=6, space="PSUM")
        ps_small = tc.alloc_tile_pool(name="ps_small", bufs=2, space="PSUM")

        ones = const_pool.tile([1, 128], BF16, tag="ones")
        nc.gpsimd.memset(ones[:], 1.0)
        ones128 = const_pool.tile([128, 1], BF16, tag="ones128")
        nc.gpsimd.memset(ones128[:], 1.0)
        # block-diag ones [2q, 128]: col j sums the q-rows of head(j)
        onesbd = const_pool.tile([128, 128], BF16, tag="onesbd")
        nc.gpsimd.memset(onesbd[:], 0.0)
        nc.gpsimd.memset(onesbd[0:64, 0:64], 1.0)
        nc.gpsimd.memset(onesbd[64:128, 64:128], 1.0)

        # first batch's X goes out before the (much larger) weight DMAs so
        # the QKV gemm can start immediately; weights stream behind. X and W
        # ship as fp8 hi/lo chunk-pair tiles [128, 2, cols] (dim 1 = the two
        # K=128 sub-chunks one DoubleRow matmul contracts).
        def load_x(b):
            tiles = []
            for c2 in range(4):
                x4 = sb.tile([128, 2, 2, 640], FP8, tag=f"x4{c2}", bufs=2)
                nc.sync.dma_start(x4[:], x8_d[b, c2])
                tiles.append((x4[:, :, 0, :], x4[:, :, 1, :]))
            return tiles

        XT0 = load_x(0)

        wh, wl = [], []
        w4s = []
        for c2 in range(4):
            w4 = w_pool.tile([128, 2, 2, 3 * C], FP8, tag=f"w4{c2}")
            w4s.append(w4)
            wh.append(w4[:, :, 0, :])
            wl.append(w4[:, :, 1, :])
        for blk in range(6):
            cs = slice(512 * blk, 512 * (blk + 1))
            for c2 in range(4):
                nc.sync.dma_start(w4s[c2][:, :, :, cs], w8_d[c2, :, :, :, cs])
        wph, wpl = [], []
        for c2 in range(4):
            t = w_pool.tile([128, 2, 2, C], FP8, tag=f"wp4{c2}")
            nc.sync.dma_start(t[:], wp8_d[c2])
            wph.append(t[:, :, 0, :])
            wpl.append(t[:, :, 1, :])
        wb = w_pool.tile([1, C], BF16, tag="wb")
        nc.sync.dma_start(wb[:], wbias_d[:])

        # persistent zeros: these tiles only ever get their "active" region
        # rewritten, so zero every rotation buffer once up front and never
        # memset in the loop.
        for _ in range(2):
            for j in range(8):
                qp = sb.tile([128, 128], BF16, tag=f"qp{j}")
                nc.gpsimd.memset(qp[:], 0.0)
            for pp in range(8):
                qd = sb.tile([128, 128], BF16, tag=f"qd{pp}")
                nc.gpsimd.memset(qd[:], 0.0)
        # (row 64 = token 576 is real and rewritten every batch; rows 65+ are
        # pad and must stay finite-zero. Partition ranges must start aligned,
        # so zero [64:128] once -- the loop re-writes row 64 before reading.)
        for _ in range(3):
            a1t = sb.tile([128, 5, 128], BF16, tag="a1t5", bufs=3)
            nc.gpsimd.memset(a1t[64:128, 4, :], 0.0)
        for _ in range(2):
            vt = sb.tile([128, C], BF16, tag="v4", bufs=2)
            nc.gpsimd.memset(vt[64:128, :], 0.0)

        # bias broadcast [128, 1024]; built lazily (first use is phase 8)
        bias = const_pool.tile([128, C], BF16, tag="bias")
        bias_built = [False]

        def build_bias():
            if bias_built[0]:
                return
            bias_built[0] = True
            for half in range(2):
                cs = slice(512 * half, 512 * (half + 1))
                bps = ps_big.tile([128, 512], F32, tag="pbig")
                nc.tensor.matmul(
                    bps[:], ones[0:1, :], wb[0:1, cs], start=True, stop=True
                )
                nc.any.tensor_copy(bias[:, cs], bps[:])

        # ---- front: QKV gemm + V + pooling for one batch, as thunks so the
        # emission can interleave into the previous batch's middle ----
        def front_thunks(b, XT):
            qkT = []
            V = []
            QpBD = []

            def phase2_chunk(cc):
                # q/k channels transposed: qkT [c, n] = 32 * true
                qt = sb.tile([128, 640], BF16, tag=f"qkt{cc}", bufs=2)
                csl = slice(128 * cc, 128 * (cc + 1))
                for ci, (n0, nw) in enumerate(NF):
                    ps = (ps_big if nw > 128 else ps_small).tile(
                        [128, nw], F32, tag="pbig" if nw > 128 else "psmall"
                    )
                    terms = (
                        [(wh[c2], XT[c2][0]) for c2 in range(4)]
                        + [(wl[c2], XT[c2][0]) for c2 in range(4)]
                        + [(wh[c2], XT[c2][1]) for c2 in range(4)]
                    )
                    for ti, (wt, xt_) in enumerate(terms):
                        nc.tensor.matmul(
                            ps[:],
                            wt[:, :, csl],
                            xt_[:, :, n0 : n0 + nw],
                            start=(ti == 0),
                            stop=(ti == 11),
                            perf_mode=DR,
                        )
                    if (cc + ci) % 2 == 0:
                        nc.vector.tensor_copy(qt[:, n0 : n0 + nw], ps[:])
                    else:
                        nc.scalar.copy(qt[:, n0 : n0 + nw], ps[:])
                qkT.append(qt)

            def phase3_tile(t):
                # V natural layout [n, c] = 32 * true; pad rows stay zero
                # because X pad columns are zero in both hi and lo.
                toff, rows = TOK[t]
                vt = sb.tile([128, C], BF16, tag=f"v{t}", bufs=2)
                lo = toff if rows == 128 else 512
                for h2 in range(2):
                    ps = ps_big.tile([128, 512], F32, tag="pbig")
                    cs = slice(2048 + 512 * h2, 2048 + 512 * (h2 + 1))
                    terms = (
                        [(XT[c2][0], wh[c2]) for c2 in range(4)]
                        + [(XT[c2][0], wl[c2]) for c2 in range(4)]
                        + [(XT[c2][1], wh[c2]) for c2 in range(4)]
                    )
                    for ti, (xt_, wt) in enumerate(terms):
                        nc.tensor.matmul(
                            ps[0:rows, :],
                            xt_[:, :, lo : lo + rows],
                            wt[:, :, cs],
                            start=(ti == 0),
                            stop=(ti == 11),
                            perf_mode=DR,
                        )
                    nc.scalar.copy(vt[0:rows, 512 * h2 : 512 * (h2 + 1)], ps[0:rows, :])
                V.append(vt)

            def pool_j(j):
                # pooled queries, block-diag [c, 2q] per pair; qkT carries
                # 32x, so alpha makes qp = SCALE * Qp_true / 32 which renders
                # both score matmuls exact.
                qsum = sb.tile([128, 64], F32, tag="qsum", bufs=3)
                view = qkT[j][:, 0:576].rearrange(
                    "p (pr dr pc dc) -> p pr pc dr dc", pr=8, dr=3, pc=8, dc=3
                )
                nc.vector.reduce_sum(qsum[:], view, axis=mybir.AxisListType.XY)
                alpha = SCALE / (9.0 * WSCALE * WSCALE)
                qp = sb.tile([128, 128], BF16, tag=f"qp{j}")
                nc.gpsimd.tensor_scalar_mul(qp[0:64, 0:64], qsum[0:64, :], alpha)
                nc.gpsimd.tensor_scalar_mul(qp[64:128, 64:128], qsum[64:128, :], alpha)
                QpBD.append(qp)

            thunks = [lambda cc=cc: phase2_chunk(cc) for cc in range(16)]
            thunks += [lambda t=t: phase3_tile(t) for t in range(5)]
            thunks += [lambda j=j: pool_j(j) for j in range(8)]
            return thunks, (qkT, V, QpBD)

        # ---- back: attention middle + output projection for one batch ----
        def back_emit(b, state, filler):
            qkT, V, QpBD = state
            fill_i = [0]
            # small filler (last batch: only deferred phase8) is rationed
            # 1/slot so it covers the whole middle; big filler keeps 2/slot
            fq = 2 if len(filler) >= 20 else 1

            def fill(k):
                for _ in range(fq if k < 100 else k):
                    if fill_i[0] < len(filler):
                        filler[fill_i[0]]()
                        fill_i[0] += 1

            A1T = [None] * 8
            E2 = [None] * 8
            RZ = [None] * 8
            R1 = [None] * 8
            E2N = [None] * 8
            QdBD = [None] * 8
            OTH = [None] * 4
            OTL = [None] * 4

            def scores(pp):
                # s1T [n, 2q]: lhsT = qkT_k chunk, rhs = QpBD; exp rows
                # limited to real tokens (pad rows stay prologue-zero)
                pa = ps_big.tile([128, 512], F32, tag="pbig")
                for t in range(4):
                    nc.tensor.matmul(
                        pa[:, 128 * t : 128 * (t + 1)],
                        qkT[8 + pp][:, 128 * t : 128 * (t + 1)],
                        QpBD[pp][:],
                        start=True,
                        stop=True,
                    )
                pb = ps_small.tile([128, 128], F32, tag="psmall")
                nc.tensor.matmul(
                    pb[:], qkT[8 + pp][:, 512:640], QpBD[pp][:], start=True, stop=True
                )
                a1t = sb.tile([128, 5, 128], BF16, tag="a1t5", bufs=3)
                nc.scalar.activation(
                    a1t[:, 0:4, :], pa[:].rearrange("p (t q) -> p t q", t=4), EXP
                )
                nc.scalar.activation(a1t[0:65, 4, :], pb[0:65, :], EXP)
                A1T[pp] = a1t

                # s2T [2q, n] = QpBD.T @ qkT_q; exp straight to E2 bf16
                e2 = sb.tile([128, 640], BF16, tag="e2", bufs=3)
                for n0, nw in NF:
                    ps = (ps_big if nw > 128 else ps_small).tile(
                        [128, nw], F32, tag="pbig" if nw > 128 else "psmall"
                    )
                    nc.tensor.matmul(
                        ps[:], QpBD[pp][:], qkT[pp][:, n0 : n0 + nw],
                        start=True, stop=True,
                    )
                    nc.scalar.activation(e2[:, n0 : n0 + nw], ps[:], EXP)
                E2[pp] = e2

            def tails(pp):
                a1t = A1T[pp]
                # Qd_raw pair product [2q, dv] plus a Z1 ones-column, both
                # accumulated into one psum bank; diag blocks kept raw
                # (1/Z1 is folded into e2n below)
                qd_ps = ps_small.tile([128, 129], F32, tag="psmall")
                for t in range(5):
                    nc.tensor.matmul(
                        qd_ps[:, 0:128],
                        a1t[:, t, :],
                        V[t][:, 128 * pp : 128 * (pp + 1)],
                        start=(t == 0),
                        stop=(t == 4),
                    )
                for t in range(5):
                    nc.tensor.matmul(
                        qd_ps[:, 128:129],
                        a1t[:, t, :],
                        ones128[:],
                        start=(t == 0),
                        stop=(t == 4),
                    )
                r1 = sb.tile([128, 1], F32, tag="r1", bufs=3)
                nc.vector.reciprocal(r1[:], qd_ps[:, 128:129])
                R1[pp] = r1
                qd = sb.tile([128, 128], BF16, tag=f"qd{pp}")
                nc.vector.tensor_copy(qd[0:64, 0:64], qd_ps[0:64, 0:64])
                nc.vector.tensor_copy(qd[64:128, 64:128], qd_ps[64:128, 64:128])
                QdBD[pp] = qd

                # Z2 pre-broadcast over q-rows, then 1/Z2
                e2 = E2[pp]
                rz = sb.tile([128, 640], BF16, tag="rz", bufs=3)
                for n0, nw in NF:
                    zb = (ps_big if nw > 128 else ps_small).tile(
                        [128, nw], F32, tag="pbig" if nw > 128 else "psmall"
                    )
                    nc.tensor.matmul(
                        zb[:], onesbd[:], e2[:, n0 : n0 + nw], start=True, stop=True
                    )
                    with nc.allow_low_precision(reason="bf16 1/Z2 is plenty"):
                        nc.vector.reciprocal(rz[:, n0 : n0 + nw], zb[:])
                RZ[pp] = rz

                # fused: e2n = (E2 * r1[q-partition]) * (1/Z2)
                e2n = sb.tile([128, 640], BF16, tag="e2n", bufs=4)
                nc.vector.scalar_tensor_tensor(
                    e2n[:, 0:577], e2[:, 0:577], r1[:, 0:1], rz[:, 0:577],
                    op0=MULT, op1=MULT,
                )
                E2N[pp] = e2n

            def out_mm(pp):
                e2n = E2N[pp]
                oa = ps_big.tile([128, 512], F32, tag="pbig")
                ob = ps_small.tile([128, 65], F32, tag="psmall")
                nc.tensor.matmul(
                    oa[:], QdBD[pp][:], e2n[:, 0:512], start=True, stop=True
                )
                nc.tensor.matmul(
                    ob[:], QdBD[pp][:], e2n[:, 512:577], start=True, stop=True
                )
                c2, sub = pp // 2, pp % 2
                if sub == 0:
                    oth = sb.tile([128, 2, 640], FP8, tag=f"oth{c2}", bufs=2)
                    otl = sb.tile([128, 2, 640], FP8, tag=f"otl{c2}", bufs=2)
                    OTH[c2], OTL[c2] = oth, otl
                oth, otl = OTH[c2], OTL[c2]
                # hi = fp8(outT); lo = fp8(outT - hi), split across engines
                SUB = mybir.AluOpType.subtract
                nc.scalar.copy(oth[:, sub, 0:512], oa[:])
                nc.scalar.copy(oth[:, sub, 512:577], ob[:])
                nc.vector.tensor_tensor(
                    otl[:, sub, 0:512], oa[:], oth[:, sub, 0:512], op=SUB
                )
                nc.vector.tensor_tensor(
                    otl[:, sub, 512:577], ob[:], oth[:, sub, 512:577], op=SUB
                )

            for pp in range(8):
                scores(pp)
                if pp >= 1:
                    tails(pp - 1)
                if pp >= 3:
                    out_mm(pp - 3)
                fill(2)
            tails(7)
            out_mm(5)
            fill(1)
            out_mm(6)
            fill(1)
            out_mm(7)
            fill(100)
            return OTH, OTL

        # ---- output projection + bias -> bf16 -> DRAM; emitted as thunks
        # and used as stall-free PE filler for the NEXT batch's middle ----
        def phase8_thunks(b, OTH, OTL):
            build_bias()
            Y = {}

            def unit(t, half):
                toff, rows = TOK[t]
                lo = toff if rows == 128 else 512
                if half == 0:
                    Y[t] = sb.tile([128, 1024], BF16, tag="y", bufs=2, name="y")
                y = Y[t]
                cs = slice(512 * half, 512 * (half + 1))
                ps = ps_big.tile([128, 512], F32, tag="pbig")
                terms = (
                    [(OTH[c2], wph[c2]) for c2 in range(4)]
                    + [(OTL[c2], wph[c2]) for c2 in range(4)]
                    + [(OTH[c2], wpl[c2]) for c2 in range(4)]
                )
                for ti, (ot_, wt) in enumerate(terms):
                    nc.tensor.matmul(
                        ps[0:rows, :],
                        ot_[:, :, lo : lo + rows],
                        wt[:, :, cs],
                        start=(ti == 0),
                        stop=(ti == 11),
                        perf_mode=DR,
                    )
                if (t + half) % 2 == 1:
                    nc.vector.tensor_tensor(
                        y[0:rows, cs], ps[0:rows, :], bias[0:rows, cs],
                        op=mybir.AluOpType.add,
                    )
                else:
                    nc.scalar.activation(
                        y[0:rows, cs], ps[0:rows, :],
                        mybir.ActivationFunctionType.Copy,
                    )
                    nc.gpsimd.tensor_tensor(
                        y[0:rows, cs], y[0:rows, cs], bias[0:rows, cs],
                        op=mybir.AluOpType.add,
                    )
                if half == 1:
                    nc.sync.dma_start(out_d[b, toff : toff + rows, :], y[0:rows, :])

            return [
                lambda t=t, half=half: unit(t, half)
                for t in range(5)
                for half in range(2)
            ]

        repeat = int(os.environ.get("KERNEL_REPEAT", "1"))
        total = nb * repeat
        # emit batch 0's front directly; every later front interleaves into
        # the previous batch's middle as PE filler, and every batch's output
        # projection interleaves into the NEXT batch's middle.
        thunks, state = front_thunks(0, XT0)
        for th in thunks:
            th()
        ph8 = []
        for i in range(total):
            b = i % nb
            if i + 1 < total:
                XTn = load_x((i + 1) % nb)
                nxt_thunks, nxt_state = front_thunks((i + 1) % nb, XTn)
            else:
                nxt_thunks, nxt_state = [], None
            oth, otl = back_emit(b, state, ph8 + nxt_thunks)
            ph8 = phase8_thunks(b, oth, otl)
            state = nxt_state
        for th in ph8:
            th()

        for p in (ps_small, ps_big, sb, w_pool, const_pool):
            p.release()

    nc.compile()
    return nc


_NC_CACHE = {}


def _get_nc(nb: int = NB):
    if nb not in _NC_CACHE:
        _NC_CACHE[nb] = build_program(nb)
    return _NC_CACHE[nb]


def _split_fp8(a):
    """e4m3 value + e4m3 residual (round-to-nearest both times)."""
    hi = a.astype(ml_dtypes.float8_e4m3)
    lo = (a - hi.astype(np.float32)).astype(ml_dtypes.float8_e4m3)
    return hi, lo


def kernel(X, W_qkv, W_proj, b_proj, layer_idx=None):
    assert X.shape == (B, N, C)
    nc = _get_nc(NB)
    xt = np.zeros((B, C, 640), dtype=np.float32)
    xt[:, :, :N] = np.asarray(X, dtype=np.float32).transpose(0, 2, 1)
    xhi, xlo = _split_fp8(xt)
    # [b, c2, p, i, v, n]
    x8 = np.ascontiguousarray(
        np.stack([xhi, xlo], axis=2)
        .reshape(B, 4, 2, 128, 2, 640)
        .transpose(0, 1, 3, 2, 4, 5)
    )
    wq32 = np.ascontiguousarray(np.asarray(W_qkv, dtype=np.float32).T) * 32.0
    whi, wlo = _split_fp8(wq32)
    # [c2, p, i, v, c]
    w8 = np.ascontiguousarray(
        np.stack([whi, wlo], axis=1)
        .reshape(4, 2, 128, 2, 3 * C)
        .transpose(0, 2, 1, 3, 4)
    )
    wp32 = np.ascontiguousarray(np.asarray(W_proj, dtype=np.float32).T) * 32.0
    wphi, wplo = _split_fp8(wp32)
    wp8 = np.ascontiguousarray(
        np.stack([wphi, wplo], axis=1)
        .reshape(4, 2, 128, 2, C)
        .transpose(0, 2, 1, 3, 4)
    )
    wbias = (np.asarray(b_proj, dtype=np.float32).reshape(1, C) * 1024.0).astype(
        ml_dtypes.bfloat16
    )
    in_maps = [
        {
            "x8": x8[NB * i : NB * (i + 1)],
            "w8": w8,
            "wp8": wp8,
            "wbias": wbias,
        }
        for i in range(N_CORES)
    ]
    res = run_bass_kernel_spmd(nc, in_maps, core_ids=list(range(N_CORES)))
    out = np.concatenate([res.results[i]["out"] for i in range(N_CORES)], axis=0)
    return out.astype(np.float32) / 1024.0


if __name__ == "__main__":
    rng = np.random.default_rng(0)
    X = rng.standard_normal((B, N, C), dtype=np.float32)
    W_qkv = rng.standard_normal((3 * C, C), dtype=np.float32) * C**-0.5
    W_proj = rng.standard_normal((C, C), dtype=np.float32) * C**-0.5
    b_proj = np.zeros(C, dtype=np.float32)
    out = kernel(X, W_qkv, W_proj, b_proj, 1)
    print(out.shape, out.dtype)


# revision 48
# speedup vs baseline: 1.0157x; 1.0157x over previous
"""Trainium2 Bass kernel for two-stage pooled-query attention.

Problem (hardcoded):
    B=32, N=577, C=1024, H=16 heads, d=64, pooled queries 8x8 (3x3 mean over
    24x24 grid of non-cls tokens).
    qkv = X @ W_qkv.T ; pool Xq -> Qp ; s1 = softmax(Qp*s @ K^T) @ V ;
    s2 = softmax(Xq*s @ Qp^T) @ s1 ; out = s2 @ W_proj.T + b_proj

Strategy: pure data-parallel over batch across 8 NeuronCores (4 batches per
core, no collectives). The big QKV GEMM runs fp8 DoubleRow (K=256 per
instruction at 0.5 cycles/row) with a 3-term hi/lo hybrid split to hold
accuracy:
  X @ W ~= X_hi@W_hi + X_lo@W_hi + X_hi@W_lo, each operand split on the host
  into an e4m3 value plus an e4m3 residual (W_qkv pre-scaled by 32 so its
  residual clears the fp8 subnormal floor; the 32 cancels through the pooled
  query scale and W_proj/32).

The attention middle runs bf16 with no PE transposes:
  - Stage 1 is computed transposed, s1T [n, 2q] = qkT_k.T @ QpBD, so the
    A1 weights land directly in the lhsT layout Qd needs. The softmax
    denominator Z1 comes from a 1-column ones matmul (~free), and 1/Z1 is
    folded per-q-partition into stage 2's weights instead of into Qd.
  - Stage 2 is computed transposed too, E2T [2q, n] = exp(QpBD.T @ qkT_q),
    the per-(token,head) denominator is produced pre-broadcast by one
    block-diag-ones matmul, and a single fused scalar_tensor_tensor gives
    e2n = E2T * r1[q] * (1/Z2) -- both normalizations in one op. The output
    outT [c, n] = QdBD_raw @ e2n needs no A2 transpose either.

Batches are software-pipelined: batch b+1's QKV GEMM instructions are
emitted interleaved into batch b's attention middle, so the in-order PE
queue always has independent work while the middle waits on softmax.
"""

import os
import sys

import numpy as np

sys.path.insert(0, "/opt/trn_rl_repo")

import ml_dtypes  # noqa: E402

import concourse.tile as tile  # noqa: E402
from concourse import bacc, mybir  # noqa: E402
from concourse.bass_utils import run_bass_kernel_spmd  # noqa: E402

B, N, C = 32, 577, 1024
H, D = 16, 64
SCALE = D ** -0.5
N_CORES = 8
NB = B // N_CORES  # batches per core

BF16 = mybir.dt.bfloat16
F32 = mybir.dt.float32
FP8 = mybir.dt.float8e4
DR = mybir.MatmulPerfMode.DoubleRow
WSCALE = 32.0  # host pre-scale on W_qkv^T (keeps fp8 residuals normal)
MULT = mybir.AluOpType.mult

# token chunks of 577 = 4*128 + 65
TOK = [(0, 128), (128, 128), (256, 128), (384, 128), (512, 65)]
# free-dim chunks of 577 for wide matmuls: one full psum bank + a stub
NF = [(0, 512), (512, 65)]
EXP = mybir.ActivationFunctionType.Exp


def build_program(nb: int = NB):
    nc = bacc.Bacc("TRN2", target_bir_lowering=False, debug=False)

    # host pre-arranges operands into the exact SBUF tile layout
    # [c2, p, i(sub-chunk), v(hi/lo), cols] so every DMA is contiguous
    x8_d = nc.dram_tensor("x8", [nb, 4, 128, 2, 2, 640], FP8, kind="ExternalInput")
    w8_d = nc.dram_tensor("w8", [4, 128, 2, 2, 3 * C], FP8, kind="ExternalInput")
    wp8_d = nc.dram_tensor("wp8", [4, 128, 2, 2, C], FP8, kind="ExternalInput")
    wbias_d = nc.dram_tensor("wbias", [1, C], BF16, kind="ExternalInput")
    out_d = nc.dram_tensor("out", [nb, N, C], BF16, kind="ExternalOutput")

    with tile.TileContext(nc) as tc:
        const_pool = tc.alloc_tile_pool(name="const", bufs=1)
        w_pool = tc.alloc_tile_pool(name="w", bufs=1)
        sb = tc.alloc_tile_pool(name="sb", bufs=2)
        ps_big = tc.alloc_tile_pool(name="ps_big", bufs=5, space="PSUM")
        ps_small = tc.alloc_tile_pool(name="ps_small", bufs=3, space="PSUM")

        ones = const_pool.tile([1, 128], BF16, tag="ones")
        nc.gpsimd.memset(ones[:], 1.0)
        ones128 = const_pool.tile([128, 1], BF16, tag="ones128")
        nc.gpsimd.memset(ones128[:], 1.0)
        # block-diag ones [2q, 128]: col j sums the q-rows of head(j)
        onesbd = const_pool.tile([128, 128], BF16, tag="onesbd")
        nc.gpsimd.memset(onesbd[:], 0.0)
        nc.gpsimd.memset(onesbd[0:64, 0:64], 1.0)
        nc.gpsimd.memset(onesbd[64:128, 64:128], 1.0)

        # first batch's X goes out before the (much larger) weight DMAs so
        # the QKV gemm can start immediately; weights stream behind. X and W
        # ship as fp8 hi/lo chunk-pair tiles [128, 2, cols] (dim 1 = the two
        # K=128 sub-chunks one DoubleRow matmul contracts).
        def load_x(b):
            tiles = []
            for c2 in range(4):
                x4 = sb.tile([128, 2, 2, 640], FP8, tag=f"x4{c2}", bufs=2)
                nc.sync.dma_start(x4[:], x8_d[b, c2])
                tiles.append((x4[:, :, 0, :], x4[:, :, 1, :]))
            return tiles

        XT0 = load_x(0)

        wh, wl = [], []
        w4s = []
        for c2 in range(4):
            w4 = w_pool.tile([128, 2, 2, 3 * C], FP8, tag=f"w4{c2}")
            w4s.append(w4)
            wh.append(w4[:, :, 0, :])
            wl.append(w4[:, :, 1, :])
        for blk in range(6):
            cs = slice(512 * blk, 512 * (blk + 1))
            for c2 in range(4):
                nc.sync.dma_start(w4s[c2][:, :, :, cs], w8_d[c2, :, :, :, cs])
        wph, wpl = [], []
        for c2 in range(4):
            t = w_pool.tile([128, 2, 2, C], FP8, tag=f"wp4{c2}")
            nc.sync.dma_start(t[:], wp8_d[c2])
            wph.append(t[:, :, 0, :])
            wpl.append(t[:, :, 1, :])
        wb = w_pool.tile([1, C], BF16, tag="wb")
        nc.sync.dma_start(wb[:], wbias_d[:])

        # persistent zeros: these tiles only ever get their "active" region
        # rewritten, so zero every rotation buffer once up front and never
        # memset in the loop.
        for _ in range(2):
            for j in range(8):
                qp = sb.tile([128, 128], BF16, tag=f"qp{j}")
                nc.gpsimd.memset(qp[:], 0.0)
            for pp in range(8):
                qd = sb.tile([128, 128], BF16, tag=f"qd{pp}")
                nc.gpsimd.memset(qd[:], 0.0)
        # (row 64 = token 576 is real and rewritten every batch; rows 65+ are
        # pad and must stay finite-zero. Partition ranges must start aligned,
        # so zero [64:128] once -- the loop re-writes row 64 before reading.)
        for _ in range(3):
            a1t = sb.tile([128, 5, 128], BF16, tag="a1t5", bufs=3)
            nc.gpsimd.memset(a1t[64:128, 4, :], 0.0)
        for _ in range(2):
            vt = sb.tile([128, C], BF16, tag="v4", bufs=2)
            nc.gpsimd.memset(vt[64:128, :], 0.0)

        # bias broadcast [128, 1024]; built lazily (first use is phase 8)
        bias = const_pool.tile([128, C], BF16, tag="bias")
        bias_built = [False]

        def build_bias():
            if bias_built[0]:
                return
            bias_built[0] = True
            for half in range(2):
                cs = slice(512 * half, 512 * (half + 1))
                bps = ps_big.tile([128, 512], F32, tag="pbig")
                nc.tensor.matmul(
                    bps[:], ones[0:1, :], wb[0:1, cs], start=True, stop=True
                )
                nc.any.tensor_copy(bias[:, cs], bps[:])

        # ---- front: QKV gemm + V + pooling for one batch, as thunks so the
        # emission can interleave into the previous batch's middle ----
        def front_thunks(b, XT):
            qkT = []
            V = []
            QpBD = []

            def phase2_chunk(cc):
                # q/k channels transposed: qkT [c, n] = 32 * true
                qt = sb.tile([128, 640], BF16, tag=f"qkt{cc}", bufs=2)
                csl = slice(128 * cc, 128 * (cc + 1))
                for ci, (n0, nw) in enumerate(NF):
                    ps = (ps_big if nw > 128 else ps_small).tile(
                        [128, nw], F32, tag="pbig" if nw > 128 else "psmall"
                    )
                    terms = (
                        [(wh[c2], XT[c2][0]) for c2 in range(4)]
                        + [(wl[c2], XT[c2][0]) for c2 in range(4)]
                        + [(wh[c2], XT[c2][1]) for c2 in range(4)]
                    )
                    for ti, (wt, xt_) in enumerate(terms):
                        nc.tensor.matmul(
                            ps[:],
                            wt[:, :, csl],
                            xt_[:, :, n0 : n0 + nw],
                            start=(ti == 0),
                            stop=(ti == 11),
                            perf_mode=DR,
                        )
                    if (cc + ci) % 2 == 0:
                        nc.vector.tensor_copy(qt[:, n0 : n0 + nw], ps[:])
                    else:
                        nc.scalar.copy(qt[:, n0 : n0 + nw], ps[:])
                qkT.append(qt)

            def phase3_tile(t):
                # V natural layout [n, c] = 32 * true; pad rows stay zero
                # because X pad columns are zero in both hi and lo.
                toff, rows = TOK[t]
                vt = sb.tile([128, C], BF16, tag=f"v{t}", bufs=2)
                lo = toff if rows == 128 else 512
                for h2 in range(2):
                    ps = ps_big.tile([128, 512], F32, tag="pbig")
                    cs = slice(2048 + 512 * h2, 2048 + 512 * (h2 + 1))
                    terms = (
                        [(XT[c2][0], wh[c2]) for c2 in range(4)]
                        + [(XT[c2][0], wl[c2]) for c2 in range(4)]
                        + [(XT[c2][1], wh[c2]) for c2 in range(4)]
                    )
                    for ti, (xt_, wt) in enumerate(terms):
                        nc.tensor.matmul(
                            ps[0:rows, :],
                            xt_[:, :, lo : lo + rows],
                            wt[:, :, cs],
                            start=(ti == 0),
                            stop=(ti == 11),
                            perf_mode=DR,
                        )
                    nc.scalar.copy(vt[0:rows, 512 * h2 : 512 * (h2 + 1)], ps[0:rows, :])
                V.append(vt)

            def pool_j(j):
                # pooled queries, block-diag [c, 2q] per pair; qkT carries
                # 32x, so alpha makes qp = SCALE * Qp_true / 32 which renders
                # both score matmuls exact.
                qsum = sb.tile([128, 64], F32, tag="qsum", bufs=3)
                view = qkT[j][:, 0:576].rearrange(
                    "p (pr dr pc dc) -> p pr pc dr dc", pr=8, dr=3, pc=8, dc=3
                )
                nc.vector.reduce_sum(qsum[:], view, axis=mybir.AxisListType.XY)
                alpha = SCALE / (9.0 * WSCALE * WSCALE)
                qp = sb.tile([128, 128], BF16, tag=f"qp{j}")
                nc.gpsimd.tensor_scalar_mul(qp[0:64, 0:64], qsum[0:64, :], alpha)
                nc.gpsimd.tensor_scalar_mul(qp[64:128, 64:128], qsum[64:128, :], alpha)
                QpBD.append(qp)

            thunks = [lambda cc=cc: phase2_chunk(cc) for cc in range(16)]
            thunks += [lambda t=t: phase3_tile(t) for t in range(5)]
            thunks += [lambda j=j: pool_j(j) for j in range(8)]
            return thunks, (qkT, V, QpBD)

        # ---- back: attention middle + output projection for one batch ----
        def back_emit(b, state, filler):
            qkT, V, QpBD = state
            fill_i = [0]
            # small filler (last batch: only deferred phase8) is rationed
            # 1/slot so it covers the whole middle; big filler keeps 2/slot
            fq = 2 if len(filler) >= 20 else 1

            def fill(k):
                for _ in range(fq if k < 100 else k):
                    if fill_i[0] < len(filler):
                        filler[fill_i[0]]()
                        fill_i[0] += 1

            A1T = [None] * 8
            E2 = [None] * 8
            RZ = [None] * 8
            R1 = [None] * 8
            E2N = [None] * 8
            QdBD = [None] * 8
            OTH = [None] * 4
            OTL = [None] * 4

            def scores(pp):
                # s1T [n, 2q]: lhsT = qkT_k chunk, rhs = QpBD; exp rows
                # limited to real tokens (pad rows stay prologue-zero)
                pa = ps_big.tile([128, 512], F32, tag="pbig")
                for t in range(4):
                    nc.tensor.matmul(
                        pa[:, 128 * t : 128 * (t + 1)],
                        qkT[8 + pp][:, 128 * t : 128 * (t + 1)],
                        QpBD[pp][:],
                        start=True,
                        stop=True,
                    )
                pb = ps_small.tile([128, 128], F32, tag="psmall")
                nc.tensor.matmul(
                    pb[:], qkT[8 + pp][:, 512:640], QpBD[pp][:], start=True, stop=True
                )
                a1t = sb.tile([128, 5, 128], BF16, tag="a1t5", bufs=3)
                nc.scalar.activation(
                    a1t[:, 0:4, :], pa[:].rearrange("p (t q) -> p t q", t=4), EXP
                )
                nc.scalar.activation(a1t[0:65, 4, :], pb[0:65, :], EXP)
                A1T[pp] = a1t

                # s2T [2q, n] = QpBD.T @ qkT_q; exp straight to E2 bf16
                e2 = sb.tile([128, 640], BF16, tag="e2", bufs=3)
                for n0, nw in NF:
                    ps = (ps_big if nw > 128 else ps_small).tile(
                        [128, nw], F32, tag="pbig" if nw > 128 else "psmall"
                    )
                    nc.tensor.matmul(
                        ps[:], QpBD[pp][:], qkT[pp][:, n0 : n0 + nw],
                        start=True, stop=True,
                    )
                    nc.scalar.activation(e2[:, n0 : n0 + nw], ps[:], EXP)
                E2[pp] = e2

            def tails(pp):
                a1t = A1T[pp]
                # Qd_raw pair product [2q, dv] plus a Z1 ones-column, both
                # accumulated into one psum bank; diag blocks kept raw
                # (1/Z1 is folded into e2n below)
                qd_ps = ps_small.tile([128, 129], F32, tag="psmall")
                for t in range(5):
                    nc.tensor.matmul(
                        qd_ps[:, 0:128],
                        a1t[:, t, :],
                        V[t][:, 128 * pp : 128 * (pp + 1)],
                        start=(t == 0),
                        stop=(t == 4),
                    )
                for t in range(5):
                    nc.tensor.matmul(
                        qd_ps[:, 128:129],
                        a1t[:, t, :],
                        ones128[:],
                        start=(t == 0),
                        stop=(t == 4),
                    )
                r1 = sb.tile([128, 1], F32, tag="r1", bufs=3)
                nc.vector.reciprocal(r1[:], qd_ps[:, 128:129])
                R1[pp] = r1
                qd = sb.tile([128, 128], BF16, tag=f"qd{pp}")
                nc.vector.tensor_copy(qd[0:64, 0:64], qd_ps[0:64, 0:64])
                nc.vector.tensor_copy(qd[64:128, 64:128], qd_ps[64:128, 64:128])
                QdBD[pp] = qd

                # Z2 pre-broadcast over q-rows, then 1/Z2
                e2 = E2[pp]
                rz = sb.tile([128, 640], BF16, tag="rz", bufs=3)
                for n0, nw in NF:
                    zb = (ps_big if nw > 128 else ps_small).tile(
                        [128, nw], F32, tag="pbig" if nw > 128 else "psmall"
                    )
                    nc.tensor.matmul(
                        zb[:], onesbd[:], e2[:, n0 : n0 + nw], start=True, stop=True
                    )
                    with nc.allow_low_precision(reason="bf16 1/Z2 is plenty"):
                        nc.vector.reciprocal(rz[:, n0 : n0 + nw], zb[:])
                RZ[pp] = rz

                # fused: e2n = (E2 * r1[q-partition]) * (1/Z2)
                e2n = sb.tile([128, 640], BF16, tag="e2n", bufs=4)
                nc.vector.scalar_tensor_tensor(
                    e2n[:, 0:577], e2[:, 0:577], r1[:, 0:1], rz[:, 0:577],
                    op0=MULT, op1=MULT,
                )
                E2N[pp] = e2n

            def out_mm(pp):
                e2n = E2N[pp]
                oa = ps_big.tile([128, 512], F32, tag="pbig")
                ob = ps_small.tile([128, 65], F32, tag="psmall")
                nc.tensor.matmul(
                    oa[:], QdBD[pp][:], e2n[:, 0:512], start=True, stop=True
                )
                nc.tensor.matmul(
                    ob[:], QdBD[pp][:], e2n[:, 512:577], start=True, stop=True
                )
                c2, sub = pp // 2, pp % 2
                if sub == 0:
                    oth = sb.tile([128, 2, 640], FP8, tag=f"oth{c2}", bufs=2)
                    otl = sb.tile([128, 2, 640], FP8, tag=f"otl{c2}", bufs=2)
                    OTH[c2], OTL[c2] = oth, otl
                oth, otl = OTH[c2], OTL[c2]
                # hi = fp8(outT); lo = fp8(outT - hi), split across engines
                SUB = mybir.AluOpType.subtract
                nc.scalar.copy(oth[:, sub, 0:512], oa[:])
                nc.scalar.copy(oth[:, sub, 512:577], ob[:])
                nc.vector.tensor_tensor(
                    otl[:, sub, 0:512], oa[:], oth[:, sub, 0:512], op=SUB
                )
                nc.vector.tensor_tensor(
                    otl[:, sub, 512:577], ob[:], oth[:, sub, 512:577], op=SUB
                )

            for pp in range(8):
                scores(pp)
                if pp >= 1:
                    tails(pp - 1)
                if pp >= 3:
                    out_mm(pp - 3)
                fill(2)
            tails(7)
            out_mm(5)
            fill(1)
            out_mm(6)
            fill(1)
            out_mm(7)
            fill(100)
            return OTH, OTL

        # ---- output projection + bias -> bf16 -> DRAM; emitted as thunks
        # and used as stall-free PE filler for the NEXT batch's middle ----
        def phase8_thunks(b, OTH, OTL):
            build_bias()
            Y = {}

            def unit(t, half):
                toff, rows = TOK[t]
                lo = toff if rows == 128 else 512
                if half == 0:
                    Y[t] = sb.tile([128, 1024], BF16, tag="y", bufs=2, name="y")
                y = Y[t]
                cs = slice(512 * half, 512 * (half + 1))
                ps = ps_big.tile([128, 512], F32, tag="pbig")
                terms = (
                    [(OTH[c2], wph[c2]) for c2 in range(4)]
                    + [(OTL[c2], wph[c2]) for c2 in range(4)]
                    + [(OTH[c2], wpl[c2]) for c2 in range(4)]
                )
                for ti, (ot_, wt) in enumerate(terms):
                    nc.tensor.matmul(
                        ps[0:rows, :],
                        ot_[:, :, lo : lo + rows],
                        wt[:, :, cs],
                        start=(ti == 0),
                        stop=(ti == 11),
                        perf_mode=DR,
                    )
                if (t + half) % 2 == 1:
                    nc.vector.tensor_tensor(
                        y[0:rows, cs], ps[0:rows, :], bias[0:rows, cs],
                        op=mybir.AluOpType.add,
                    )
                else:
                    nc.scalar.activation(
                        y[0:rows, cs], ps[0:rows, :],
                        mybir.ActivationFunctionType.Copy,
                    )
                    nc.gpsimd.tensor_tensor(
                        y[0:rows, cs], y[0:rows, cs], bias[0:rows, cs],
                        op=mybir.AluOpType.add,
                    )
                if half == 1:
                    nc.sync.dma_start(out_d[b, toff : toff + rows, :], y[0:rows, :])

            return [
                lambda t=t, half=half: unit(t, half)
                for t in range(5)
                for half in range(2)
            ]

        repeat = int(os.environ.get("KERNEL_REPEAT", "1"))
        total = nb * repeat
        # emit batch 0's front directly; every later front interleaves into
        # the previous batch's middle as PE filler, and every batch's output
        # projection interleaves into the NEXT batch's middle.
        thunks, state = front_thunks(0, XT0)
        for th in thunks:
            th()
        ph8 = []
        for i in range(total):
            b = i % nb
            if i + 1 < total:
                XTn = load_x((i + 1) % nb)
                nxt_thunks, nxt_state = front_thunks((i + 1) % nb, XTn)
            else:
                nxt_thunks, nxt_state = [], None
            oth, otl = back_emit(b, state, ph8 + nxt_thunks)
            ph8 = phase8_thunks(b, oth, otl)
            state = nxt_state
        for th in ph8:
            th()

        for p in (ps_small, ps_big, sb, w_pool, const_pool):
            p.release()

    nc.compile()
    return nc


_NC_CACHE = {}


def _get_nc(nb: int = NB):
    if nb not in _NC_CACHE:
        _NC_CACHE[nb] = build_program(nb)
    return _NC_CACHE[nb]


def _split_fp8(a):
    """e4m3 value + e4m3 residual (round-to-nearest both times)."""
    hi = a.astype(ml_dtypes.float8_e4m3)
    lo = (a - hi.astype(np.float32)).astype(ml_dtypes.float8_e4m3)
    return hi, lo


def kernel(X, W_qkv, W_proj, b_proj, layer_idx=None):
    assert X.shape == (B, N, C)
    nc = _get_nc(NB)
    xt = np.zeros((B, C, 640), dtype=np.float32)
    xt[:, :, :N] = np.asarray(X, dtype=np.float32).transpose(0, 2, 1)
    xhi, xlo = _split_fp8(xt)
    # [b, c2, p, i, v, n]
    x8 = np.ascontiguousarray(
        np.stack([xhi, xlo], axis=2)
        .reshape(B, 4, 2, 128, 2, 640)
        .transpose(0, 1, 3, 2, 4, 5)
    )
    wq32 = np.ascontiguousarray(np.asarray(W_qkv, dtype=np.float32).T) * 32.0
    whi, wlo = _split_fp8(wq32)
    # [c2, p, i, v, c]
    w8 = np.ascontiguousarray(
        np.stack([whi, wlo], axis=1)
        .reshape(4, 2, 128, 2, 3 * C)
        .transpose(0, 2, 1, 3, 4)
    )
    wp32 = np.ascontiguousarray(np.asarray(W_proj, dtype=np.float32).T) * 32.0
    wphi, wplo = _split_fp8(wp32)
    wp8 = np.ascontiguousarray(
        np.stack([wphi, wplo], axis=1)
        .reshape(4, 2, 128, 2, C)
        .transpose(0, 2, 1, 3, 4)
    )
    wbias = (np.asarray(b_proj, dtype=np.float32).reshape(1, C) * 1024.0).astype(
        ml_dtypes.bfloat16
    )
    in_maps = [
        {
            "x8": x8[NB * i : NB * (i + 1)],
            "w8": w8,
            "wp8": wp8,
            "wbias": wbias,
        }
        for i in range(N_CORES)
    ]
    res = run_bass_kernel_spmd(nc, in_maps, core_ids=list(range(N_CORES)))
    out = np.concatenate([res.results[i]["out"] for i in range(N_CORES)], axis=0)
    return out.astype(np.float32) / 1024.0


if __name__ == "__main__":
    rng = np.random.default_rng(0)
    X = rng.standard_normal((B, N, C), dtype=np.float32)
    W_qkv = rng.standard_normal((3 * C, C), dtype=np.float32) * C**-0.5
    W_proj = rng.standard_normal((C, C), dtype=np.float32) * C**-0.5
    b_proj = np.zeros(C, dtype=np.float32)
    out = kernel(X, W_qkv, W_proj, b_proj, 1)
    print(out.shape, out.dtype)
